# revision 6
# baseline (speedup 1.0000x reference)
"""BitLinear forward kernel for Trainium2 (8 NeuronCores, data-parallel).

Computes y = sign(x) @ (alpha * code)^T + b where code/alpha are the
per-row ternarization of W (BitNet-style, delta_w = 0.05, delta_a = 0.0).

Sharding: x is split over batch*seq (16384 rows) across 8 cores; W is
replicated (each core quantizes the full W on-device); outputs are
concatenated on the host.

The matmul runs in fp8 (values are exactly {-1, 0, +1}) with fp32 PSUM
accumulation, so integer counts are exact; the per-output-feature alpha
scale is applied in fp32 on eviction. b from setup_inputs() is zeros; a
nonzero b takes a second elementwise pass.
"""

import sys

for _p in ("/opt/trn_rl_repo", "/opt/trn_rl_repo/concourse"):
    if _p not in sys.path:
        sys.path.insert(0, _p)

import numpy as np

import concourse.bass as bass
import concourse.tile as tile
import concourse.mybir as mybir
from concourse import bacc
from concourse.bass_utils import run_bass_kernel_spmd

# Problem shape (hardcoded per contract)
B, S, D, O = 4, 4096, 2048, 2048
N_CORES = 8
T = (B * S) // N_CORES  # 2048 token rows per core
DELTA_W = 0.05

P = 128
TT = T // P  # 16 t-tiles
DT = D // P  # 16 d-tiles
WT = O // P  # 16 W row-tiles
NB = 4  # psum banks per t-tile (512 f32 each)
NBW = O // NB  # 512
Q = 4  # transpose quarter blocks
QR = T // Q  # 512 rows per quarter

F32 = mybir.dt.float32
BF16 = mybir.dt.bfloat16
FP8 = mybir.dt.float8e4
U16 = mybir.dt.uint16

_CACHE = {}


def _build(with_bias: bool):
    nc = bacc.Bacc("TRN2", target_bir_lowering=False, debug=False,
                   num_devices=N_CORES)
    x_d = nc.dram_tensor("x", [T, D], F32, kind="ExternalInput").ap()
    w_d = nc.dram_tensor("W", [O, D], F32, kind="ExternalInput").ap()
    y_d = nc.dram_tensor("y", [T, O], F32, kind="ExternalOutput").ap()
    if with_bias:
        b_d = nc.dram_tensor("b", [O], F32, kind="ExternalInput").ap()

    with tile.TileContext(nc) as tc:
        with (
            tc.tile_pool(name="dram", bufs=1, space="DRAM") as dram,
            tc.tile_pool(name="wload", bufs=2) as wload,
            tc.tile_pool(name="awc", bufs=2) as awc_pool,
            tc.tile_pool(name="junk", bufs=1) as junk_pool,
            tc.tile_pool(name="wsmall", bufs=2) as wsmall,
            tc.tile_pool(name="stats", bufs=1) as stats,
            tc.tile_pool(name="xload", bufs=2) as xload,
            tc.tile_pool(name="xsign", bufs=2) as xsign,
            tc.tile_pool(name="tpose", bufs=4) as tpose,
            tc.tile_pool(name="codeT", bufs=DT * NB) as codeT_pool,
            tc.tile_pool(name="xqT", bufs=DT * Q) as xqT_pool,
            tc.tile_pool(name="psum", bufs=2, space="PSUM") as psum_pool,
            tc.tile_pool(name="yout", bufs=2) as yout,
            tc.tile_pool(name="bcast", bufs=1) as bcast,
        ):
            xq_dram = dram.tile([T, D], BF16)
            code_dram = dram.tile([O, D], BF16)
            alpha_dram = dram.tile([O], F32)

            # Per-row stats, one column per W row-tile
            S_all = stats.tile([P, WT], F32)      # sum(W) per row
            T_all = stats.tile([P, WT], F32)      # sum(|W - mean|) per row
            den_all = stats.tile([P, WT], F32)    # count(|Wc| >= thr) per row
            smin_all = stats.tile([P, WT], F32)   # sum(min(|Wc|, thr)) per row
            negmean_all = stats.tile([P, WT], F32)
            thr_all = stats.tile([P, WT], F32)
            alpha_all = stats.tile([P, WT], F32)

            act_junk = junk_pool.tile([P, D], F32, tag="act_junk")

            # ---------------- W quantization + x sign pipelines ----------
            for i in range(max(WT, TT)):
                if i < WT:
                    wi = i
                    wt = wload.tile([P, D], F32)
                    nc.sync.dma_start(wt[:], w_d[wi * P:(wi + 1) * P, :])

                    # S = sum(W) via ACT Copy with accumulate
                    nc.scalar.activation(
                        out=act_junk[:], in_=wt[:],
                        func=mybir.ActivationFunctionType.Copy,
                        accum_out=S_all[:, wi:wi + 1],
                    )
                    # negmean = -S/D  (1/D is a power of two: exact)
                    nc.vector.tensor_scalar_mul(
                        negmean_all[:, wi:wi + 1], S_all[:, wi:wi + 1],
                        -1.0 / D,
                    )
                    # aWc = |W - mean|, T = sum(aWc)
                    aWc = awc_pool.tile([P, D], F32)
                    nc.scalar.activation(
                        out=aWc[:], in_=wt[:],
                        func=mybir.ActivationFunctionType.Abs,
                        bias=negmean_all[:, wi:wi + 1],
                        accum_out=T_all[:, wi:wi + 1],
                    )
                    # thr = DELTA_W * T / D  (DELTA_W/D = 0.05 * 2^-11: the
                    # f32 constant equals ref's fl(0.05)*2^-11 exactly)
                    nc.vector.tensor_scalar_mul(
                        thr_all[:, wi:wi + 1], T_all[:, wi:wi + 1],
                        DELTA_W / D,
                    )
                    # sgn = Sign(W - mean) in bf16
                    sgn = wsmall.tile([P, D], BF16, tag="sgn")
                    nc.scalar.activation(
                        out=sgn[:], in_=wt[:],
                        func=mybir.ActivationFunctionType.Sign,
                        bias=negmean_all[:, wi:wi + 1],
                    )
                    # s01 = (aWc >= thr), den = count
                    s01 = wsmall.tile([P, D], BF16, tag="s01")
                    nc.vector.tensor_scalar(
                        out=s01[:], in0=aWc[:],
                        scalar1=thr_all[:, wi:wi + 1], scalar2=0.0,
                        op0=mybir.AluOpType.is_ge,
                        op1=mybir.AluOpType.add,
                        accum_out=den_all[:, wi:wi + 1],
                    )
                    # Smin = sum(min(aWc, thr))  (junk elementwise output)
                    minjunk = junk_pool.tile([P, D], F32, tag="minjunk")
                    nc.vector.tensor_scalar(
                        out=minjunk[:], in0=aWc[:],
                        scalar1=thr_all[:, wi:wi + 1], scalar2=0.0,
                        op0=mybir.AluOpType.min,
                        op1=mybir.AluOpType.add,
                        accum_out=smin_all[:, wi:wi + 1],
                    )
                    # code = sgn * s01 in bf16, then to DRAM bounce
                    code = wsmall.tile([P, D], BF16, tag="code")
                    nc.vector.tensor_mul(code[:], sgn[:], s01[:])
                    nc.sync.dma_start(
                        code_dram[wi * P:(wi + 1) * P, :], code[:])

                if i < TT:
                    ti = i
                    xb = xload.tile([P, D], BF16)
                    nc.gpsimd.dma_start(
                        xb[:], x_d[ti * P:(ti + 1) * P, :])  # f32->bf16 cast
                    xq = xsign.tile([P, D], BF16)
                    # sign(x): (x & 0x8000) | 0x3F80 -> exactly +-1.0
                    nc.vector.tensor_scalar(
                        out=xq.bitcast(U16)[:], in0=xb.bitcast(U16)[:],
                        scalar1=0x8000, scalar2=0x3F80,
                        op0=mybir.AluOpType.bitwise_and,
                        op1=mybir.AluOpType.bitwise_or,
                    )
                    nc.sync.dma_start(xq_dram[ti * P:(ti + 1) * P, :], xq[:])

            # ---------------- alpha = (T - Smin + thr*den) / max(den,1) --
            num = stats.tile([P, WT], F32, tag="num")
            nc.vector.tensor_mul(num[:], thr_all[:], den_all[:])
            nc.vector.tensor_add(num[:], num[:], T_all[:])
            nc.vector.tensor_sub(num[:], num[:], smin_all[:])
            denc = stats.tile([P, WT], F32, tag="denc")
            nc.vector.tensor_scalar_max(denc[:], den_all[:], 1.0)
            rden = stats.tile([P, WT], F32, tag="rden")
            nc.vector.reciprocal(rden[:], denc[:])
            nc.vector.tensor_mul(alpha_all[:], num[:], rden[:])
            # scatter alpha columns to DRAM [O] then broadcast-load
            nc.sync.dma_start(
                alpha_dram.rearrange("(w p) -> p w", p=P)[:, :], alpha_all[:])
            alphaB = bcast.tile([P, O], F32, tag="alphaB")
            nc.gpsimd.dma_start(
                alphaB[:], alpha_dram.unsqueeze(0).to_broadcast((P, O)))
            if with_bias:
                biasB = bcast.tile([P, O], F32, tag="biasB")
                nc.gpsimd.dma_start(
                    biasB[:], b_d.unsqueeze(0).to_broadcast((P, O)))

            # ---------------- transposed reads + fp8 conversion ----------
            # codeT quarter tiles: codeT8[di][o4] = [128 d, 512 o] fp8
            codeT8 = [[None] * NB for _ in range(DT)]
            xqT8 = [[None] * Q for _ in range(DT)]
            for di in range(DT):
                for o4 in range(NB):
                    tb = tpose.tile([P, NBW], BF16, tag="tp_code")
                    nc.sync.dma_start_transpose(
                        tb[:],
                        code_dram[o4 * NBW:(o4 + 1) * NBW,
                                  di * P:(di + 1) * P],
                    )
                    t8 = codeT_pool.tile([P, NBW], FP8, tag="codeT")
                    nc.vector.tensor_copy(out=t8[:], in_=tb[:])
                    codeT8[di][o4] = t8
                for q in range(Q):
                    tb = tpose.tile([P, QR], BF16, tag="tp_xq")
                    nc.sync.dma_start_transpose(
                        tb[:],
                        xq_dram[q * QR:(q + 1) * QR, di * P:(di + 1) * P],
                    )
                    t8 = xqT_pool.tile([P, QR], FP8, tag="xqT")
                    nc.vector.tensor_copy(out=t8[:], in_=tb[:])
                    xqT8[di][q] = t8

            # ---------------- main matmul: y[t,o] = sum_d xq^T.T @ codeT --
            for ti in range(TT):
                ps = psum_pool.tile([P, O], F32)
                q, r = divmod(ti, Q // 1)  # quarter index, offset in quarter
                q = ti // (TT // Q)
                r = ti % (TT // Q)
                for di in range(DT):
                    lhsT = xqT8[di][q][:, r * P:(r + 1) * P]
                    for o4 in range(NB):
                        nc.tensor.matmul(
                            ps[:, o4 * NBW:(o4 + 1) * NBW],
                            lhsT,
                            codeT8[di][o4][:],
                            start=(di == 0),
                            stop=(di == DT - 1),
                        )
                ysb = yout.tile([P, O], F32)
                nc.vector.tensor_mul(ysb[:], ps[:], alphaB[:])
                if with_bias:
                    nc.vector.tensor_add(ysb[:], ysb[:], biasB[:])
                nc.sync.dma_start(y_d[ti * P:(ti + 1) * P, :], ysb[:])

    nc.compile()
    return nc


def _get_nc(with_bias: bool):
    key = with_bias
    if key not in _CACHE:
        _CACHE[key] = _build(with_bias)
    return _CACHE[key]


def kernel(x: np.ndarray, W: np.ndarray, b: np.ndarray) -> np.ndarray:
    x = np.asarray(x, dtype=np.float32)
    W = np.ascontiguousarray(W, dtype=np.float32)
    b = np.asarray(b, dtype=np.float32)
    with_bias = bool(np.any(b))

    nc = _get_nc(with_bias)

    xf = np.ascontiguousarray(x.reshape(B * S, D))
    in_maps = []
    for c in range(N_CORES):
        m = {"x": np.ascontiguousarray(xf[c * T:(c + 1) * T]), "W": W}
        if with_bias:
            m["b"] = b
        in_maps.append(m)

    res = run_bass_kernel_spmd(nc, in_maps, core_ids=list(range(N_CORES)))
    y = np.concatenate([res.results[c]["y"] for c in range(N_CORES)], axis=0)
    return np.ascontiguousarray(y.reshape(B, S, O))


if __name__ == "__main__":
    rng = np.random.default_rng(0)
    x = rng.standard_normal((B, S, D), dtype=np.float32)
    W = rng.standard_normal((O, D), dtype=np.float32) * 0.03
    b = np.zeros((O,), dtype=np.float32)
    y = kernel(x, W, b)
    print("kernel ran, y shape", y.shape, "mean|y|", np.abs(y).mean())
